# revision 3
# baseline (speedup 1.0000x reference)
"""Trainium2 Bass kernel for DPMultiheadAttention (L=2048, B=2, E=1024, H=16).

Sharding: batch*head parallel across 8 cores. Core c handles batch c%2 and
heads [4*(c//2), 4*(c//2)+4). Each core computes q/k/v projections for its
256-feature slice, per-head attention, and a partial out-projection; the host
sums the per-batch partials.

Software-pipelined schedule (v2): the kernel is one long interleaved stream
instead of serial phases. The exp stream on the Scalar engine is the
second-longest resource (~136us busy) after PE (~164us), so:
  - xk/xq/xv are DMAed in parallel on the three DGE queues (scalar/sync/
    gpsimd) so K0/Q0 projections finish ~25us in and the first scores/exp
    start there, not at 80us.
  - Attention runs as 8 windows (qh, pair, head): scores -> exp -> ctx per
    128-key chunk j. Remaining projection work (V, K1/Q1) and the qh0
    out-projection are interleaved into the windows as PE filler so the PE
    queue never idles while the Scalar engine works through the exps.
  - PSUM: scores double-buffer (4 banks) + one ctx accumulator (2 banks) +
    a shared 2-buf pool for projections/out-proj (2 banks) = 8 banks.
  - Softmax denominators ride as a ones-column in the padded V operand;
    normalization (reciprocal spread over 4 partitions, DMA row-broadcast)
    happens per window off the critical path; out-projection per 128-token
    chunk follows once all four heads' ctx for that qh are normalized.
"""

import numpy as np

import concourse.bass as bass
import concourse.tile as tile
from concourse import mybir
from concourse.bass_utils import run_bass_kernel_spmd

L = 2048
B = 2
E = 1024
H = 16
D = 64
NCORES = 8
HPC = H // NCORES * B  # heads per core = 4
FL = HPC * D  # local feature slice = 256
P = 128

BF16 = mybir.dt.bfloat16
FP32 = mybir.dt.float32

TRACE = False
TRACE_KWARGS = {}
LAST_RESULTS = None


class PatchedTileContext(tile.TileContext):
    """This walrus build caps sync-wait slots per instruction at one; Tile's
    sem assigner freely attaches several. Split extra waits onto same-engine
    nops inserted just before the owning instruction."""

    MAX_WAITS = 1

    def _split_inst_waits(self, inst, out_list):
        si = getattr(inst, "sync_info", None)
        if si is not None and len(si.on_wait) > self.MAX_WAITS:
            waits = list(si.on_wait)
            keep = len(waits) - self.MAX_WAITS
            for i in range(0, keep, self.MAX_WAITS):
                out_list.append(
                    mybir.InstNoOp(
                        name=f"I-ws-{self.nc.next_id()}",
                        engine=inst.engine,
                        bass_nofuse=True,
                        sync_info=mybir.SyncInfo(
                            on_wait=waits[i : i + self.MAX_WAITS], on_update=[]
                        ),
                    )
                )
            inst.sync_info = mybir.SyncInfo(
                on_wait=waits[keep:], on_update=list(si.on_update)
            )
        out_list.append(inst)

    def _lower_ordered_insts(self, ordered):
        for insts in ordered.values():
            new_list = []
            for inst in insts:
                self._split_inst_waits(inst, new_list)
            insts[:] = new_list
        super()._lower_ordered_insts(ordered)

    def _drain_and_barrier(self, tick_clock, wait_clock):
        from bass_rust import SyncInfo
        from concourse.vector_clock import ScopedClock

        drain_inst = self.nc.sync.drain()
        wait_clock.add_sem_waits(
            drain_inst.ins, ScopedClock({None: tick_clock.global_clock})
        )
        si = drain_inst.ins.sync_info
        if si is not None and len(si.on_wait) > self.MAX_WAITS:
            waits = list(si.on_wait)
            drain_inst.ins.sync_info = SyncInfo(
                on_wait=waits[: self.MAX_WAITS], on_update=list(si.on_update)
            )
            for i in range(self.MAX_WAITS, len(waits), self.MAX_WAITS):
                nop = self.nc.sync.nop(nofuse=True)
                nop.ins.sync_info = SyncInfo(
                    on_wait=waits[i : i + self.MAX_WAITS], on_update=[]
                )

        self.nc.all_engine_barrier()
        assert self.sems is not None
        popped = self.nc._tile_sem_poison_stack.pop()
        assert popped is self._sem_poison
        self.nc.clear_and_free_semaphores(list(self.sems.allocated().values()))
        self.nc.all_engine_barrier()


def _ap3(ap, dims):
    return bass.AP(tensor=ap.tensor, offset=ap.offset, ap=dims)


def _bcast_ap(t):
    """DRAM 1-D tensor -> (128, len) partition-broadcast AP for DMA."""
    ap = t[:]
    return bass.AP(tensor=ap.tensor, offset=ap.offset, ap=[[0, P], *ap.ap])


KT = E // P  # 8 contraction tiles for projections
MT = FL // P  # 2 feature tiles (pairs)
NQ = L // 512  # 4 token chunks of 512
LT = L // P  # 16 token tiles of 128
EXPF = mybir.ActivationFunctionType.Exp


def build_nc():
    nc = bass.Bass()

    xq = nc.declare_dram_parameter("xq_t", [E, L], BF16, isOutput=False)
    xk = nc.declare_dram_parameter("xk_t", [E, L], BF16, isOutput=False)
    xv = nc.declare_dram_parameter("xv_t", [E, L], BF16, isOutput=False)
    wq = nc.declare_dram_parameter("wq_t", [E, FL], BF16, isOutput=False)
    wk = nc.declare_dram_parameter("wk_t", [E, FL], BF16, isOutput=False)
    wv = nc.declare_dram_parameter("wv_t", [E, FL], BF16, isOutput=False)
    wo = nc.declare_dram_parameter("wo_t", [FL, E], BF16, isOutput=False)
    bq = nc.declare_dram_parameter("bq", [FL], FP32, isOutput=False)
    bk = nc.declare_dram_parameter("bk", [FL], FP32, isOutput=False)
    bv = nc.declare_dram_parameter("bv", [FL], FP32, isOutput=False)
    bo = nc.declare_dram_parameter("bo", [E], FP32, isOutput=False)
    out = nc.declare_dram_parameter("out_p", [L, E], FP32, isOutput=True)

    with PatchedTileContext(nc) as tc:
        with (
            tc.tile_pool(name="singles", bufs=1) as singles,
            tc.tile_pool(name="pt", bufs=4) as pt_pool,
            tc.tile_pool(name="norm", bufs=2) as norm_pool,
            tc.tile_pool(name="outsb", bufs=2) as out_pool,
        ):
            # ---- activation-table preload: tiny exp before anything else ----
            dummy = singles.tile([1, 32], FP32, tag="dummy")
            nc.vector.memset(dummy[:], 1.0)
            nc.scalar.activation(dummy[:], dummy[:], EXPF)

            # ---- weights / biases ----
            wq_sb = singles.tile([P, KT, FL], BF16, tag="wq")
            wk_sb = singles.tile([P, KT, FL], BF16, tag="wk")
            wv_sb = singles.tile([P, KT, FL], BF16, tag="wv")
            wo_sb = singles.tile([P, MT, E], BF16, tag="wo")
            bq_sb = singles.tile([P, MT], FP32, tag="bq")
            bk_sb = singles.tile([P, MT], FP32, tag="bk")
            bv_sb = singles.tile([P, FL], FP32, tag="bv")
            bo_sb = singles.tile([P, E], FP32, tag="bo")

            # ---- activations (inputs) ----
            xq_sb = singles.tile([P, KT, L], BF16, tag="xq")
            xk_sb = singles.tile([P, KT, L], BF16, tag="xk")
            xv_sb = singles.tile([P, KT, L], BF16, tag="xv")

            # DMA: three big inputs ride three different DGE queues so they
            # land in parallel; weights (small) go first on sync.
            nc.sync.dma_start(wk_sb[:], wk.rearrange("(o p) f -> p o f", p=P))
            nc.sync.dma_start(bk_sb[:], bk.rearrange("(o p) -> p o", p=P))
            nc.sync.dma_start(wq_sb[:], wq.rearrange("(o p) f -> p o f", p=P))
            nc.sync.dma_start(bq_sb[:], bq.rearrange("(o p) -> p o", p=P))
            xk_re = xk.rearrange("(o p) m -> p o m", p=P)
            xq_re = xq.rearrange("(o p) m -> p o m", p=P)
            xv_re = xv.rearrange("(o p) m -> p o m", p=P)
            nc.scalar.dma_start(xk_sb[:, 0:4, :], xk_re[:, 0:4, :])
            nc.scalar.dma_start(xk_sb[:, 4:8, :], xk_re[:, 4:8, :])
            nc.sync.dma_start(xq_sb[:, 0:4, :], xq_re[:, 0:4, :])
            nc.sync.dma_start(xq_sb[:, 4:8, :], xq_re[:, 4:8, :])
            nc.gpsimd.dma_start(xv_sb[:], xv_re[:])
            nc.scalar.dma_start(wv_sb[:], wv.rearrange("(o p) f -> p o f", p=P))
            nc.scalar.dma_start(bv_sb[:], _bcast_ap(bv))
            nc.sync.dma_start(wo_sb[:], wo.rearrange("(o p) f -> p o f", p=P))
            nc.sync.dma_start(bo_sb[:], _bcast_ap(bo))

            # ---- persistent activations ----
            # Q^T zero-padded per head: within pair tile, head hh lives in
            # partition rows [64*hh, 64*hh+64); other rows 0.
            qtp_t = [
                singles.tile([P, 2, L], BF16, tag=f"qtp{p}", name=f"qtp{p}")
                for p in range(MT)
            ]
            kt_t = [
                singles.tile([P, L], BF16, tag=f"kt{p}", name=f"kt{p}")
                for p in range(MT)
            ]
            # V padded per head to 128 cols: [V_h (64) | ones | zeros(63)]
            v_t = [
                singles.tile([P, HPC, P], BF16, tag=f"v{j}", name=f"v{j}")
                for j in range(LT)
            ]
            ctx_t = [
                singles.tile([P, L], BF16, tag=f"ctx{p}", name=f"ctx{p}")
                for p in range(MT)
            ]

            for p in range(MT):
                nc.vector.memset(qtp_t[p][D:P, 0, :], 0.0)
                nc.vector.memset(qtp_t[p][0:D, 1, :], 0.0)
            for j in range(LT):
                nc.vector.memset(v_t[j][:, :, D : D + 1], 1.0)
                nc.vector.memset(v_t[j][:, :, D + 1 : P], 0.0)

            # ================= K0/Q0 projections (pair 0) =================
            with tc.tile_pool(name="p0", bufs=4, space="PSUM") as p0_pool:
                for which, w_sb, x_sb, b_sb in (
                    ("k0", wk_sb, xk_sb, bk_sb),
                    ("q0", wq_sb, xq_sb, bq_sb),
                ):
                    ps = [
                        p0_pool.tile([P, 512], FP32, tag="p0", name=f"ps{which}{n}")
                        for n in range(NQ)
                    ]
                    for k in range(KT):
                        for n in range(NQ):
                            nc.tensor.matmul(
                                ps[n][:],
                                w_sb[:, k, 0:P],
                                x_sb[:, k, bass.ts(n, 512)],
                                start=(k == 0),
                                stop=(k == KT - 1),
                            )
                    for n in range(NQ):
                        if which == "k0":
                            nc.vector.tensor_scalar_add(
                                kt_t[0][:, bass.ts(n, 512)], ps[n][:], b_sb[:, 0:1]
                            )
                        else:
                            nc.vector.tensor_scalar_add(
                                qtp_t[0][0:D, 0, bass.ts(n, 512)],
                                ps[n][0:D],
                                b_sb[0:D, 0:1],
                            )
                            nc.vector.tensor_scalar_add(
                                qtp_t[0][D:P, 1, bass.ts(n, 512)],
                                ps[n][D:P],
                                b_sb[D:P, 0:1],
                            )

            # ================= interleaved attention + rest =================
            with (
                tc.tile_pool(name="s_psum", bufs=2, space="PSUM") as s_pool,
                tc.tile_pool(name="c_psum", bufs=1, space="PSUM") as c_pool,
                tc.tile_pool(name="pa", bufs=2, space="PSUM") as pa_pool,
            ):

                def emit_v(lt):
                    ps = pa_pool.tile([P, 512], FP32, tag="pa", name=f"psv{lt}")
                    for k in range(KT):
                        nc.tensor.matmul(
                            ps[:, :FL],
                            xv_sb[:, k, bass.ts(lt, P)],
                            wv_sb[:, k, :],
                            start=(k == 0),
                            stop=(k == KT - 1),
                        )
                    nc.vector.tensor_add(
                        v_t[lt][:, :, 0:D],
                        ps[:, :FL].rearrange("p (h d) -> p h d", d=D),
                        bv_sb.rearrange("p (h d) -> p h d", d=D),
                    )

                def emit_kq1(which, n):
                    w_sb, x_sb, b_sb = (
                        (wk_sb, xk_sb, bk_sb) if which == "k" else (wq_sb, xq_sb, bq_sb)
                    )
                    ps = pa_pool.tile([P, 512], FP32, tag="pa", name=f"ps{which}1{n}")
                    for k in range(KT):
                        nc.tensor.matmul(
                            ps[:],
                            w_sb[:, k, bass.ts(1, P)],
                            x_sb[:, k, bass.ts(n, 512)],
                            start=(k == 0),
                            stop=(k == KT - 1),
                        )
                    if which == "k":
                        nc.vector.tensor_scalar_add(
                            kt_t[1][:, bass.ts(n, 512)], ps[:], b_sb[:, 1:2]
                        )
                    else:
                        nc.vector.tensor_scalar_add(
                            qtp_t[1][0:D, 0, bass.ts(n, 512)],
                            ps[0:D],
                            b_sb[0:D, 1:2],
                        )
                        nc.vector.tensor_scalar_add(
                            qtp_t[1][D:P, 1, bass.ts(n, 512)],
                            ps[D:P],
                            b_sb[D:P, 1:2],
                        )

                def emit_c(lt):
                    osb = out_pool.tile([P, E], FP32, tag="osb", name=f"osb{lt}")
                    for nn in range(2):
                        ps = pa_pool.tile(
                            [P, 512], FP32, tag="pa", name=f"psc{lt}_{nn}"
                        )
                        for kt_i in range(MT):
                            nc.tensor.matmul(
                                ps[:],
                                ctx_t[kt_i][:, bass.ts(lt, P)],
                                wo_sb[:, kt_i, bass.ts(nn, 512)],
                                start=(kt_i == 0),
                                stop=(kt_i == MT - 1),
                            )
                        nc.vector.tensor_add(
                            osb[:, bass.ts(nn, 512)], ps[:], bo_sb[:, bass.ts(nn, 512)]
                        )
                    nc.sync.dma_start(out[bass.ts(lt, P), :], osb[:])

                def emit_norm(qh, p, hh, cps):
                    craw = norm_pool.tile(
                        [D + 1, 1024], FP32, tag="craw", name=f"craw{qh}{p}{hh}"
                    )
                    nc.vector.tensor_copy(craw[:], cps[0 : D + 1, :])
                    # reciprocal of the sums row, spread over 4 32-aligned
                    # partitions so it runs 4 DVE lanes wide instead of 1
                    rt = norm_pool.tile([97, 256], FP32, tag="rt", name=f"rt{qh}{p}{hh}")
                    for k4 in range(4):
                        nc.vector.tensor_copy(
                            rt[32 * k4 : 32 * k4 + 1, :],
                            craw[D : D + 1, bass.ts(k4, 256)],
                        )
                    nc.vector.reciprocal(rt[:], rt[:])
                    rrow = norm_pool.tile([1, 1024], FP32, tag="rrow", name=f"rrow{qh}{p}{hh}")
                    for k4 in range(4):
                        nc.vector.tensor_copy(
                            rrow[0:1, bass.ts(k4, 256)],
                            rt[32 * k4 : 32 * k4 + 1, :],
                        )
                    rb = norm_pool.tile([D, 1024], FP32, tag="rb", name=f"rb{qh}{p}{hh}")
                    rap = rrow[0:1, :]
                    nc.sync.dma_start(
                        out=_ap3(rb[:], [rb[:].ap[0], [1, 1], rb[:].ap[1]]),
                        in_=_ap3(rap, [[1, 1], [0, D], rap.ap[-1]]),
                    )
                    nc.vector.tensor_mul(
                        ctx_t[p][D * hh : D * hh + D, bass.ds(qh * 1024, 1024)],
                        craw[0:D, :],
                        rb[:],
                    )

                def emit_window(qh, p, hh, fillers):
                    head = 2 * p + hh
                    cps = c_pool.tile(
                        [P, 1024], FP32, tag="c", name=f"cps{qh}{p}{hh}"
                    )
                    done = 0
                    nf = len(fillers)
                    for j in range(LT):
                        want = (j + 1) * nf // LT
                        while done < want:
                            fillers[done]()
                            done += 1
                        sps = s_pool.tile(
                            [P, 1024], FP32, tag="s", name=f"sps{qh}{p}{hh}_{j}"
                        )
                        for nn in range(2):
                            nc.tensor.matmul(
                                sps[:, bass.ts(nn, 512)],
                                kt_t[p][:, bass.ts(j, P)],
                                qtp_t[p][:, hh, bass.ds(qh * 1024 + nn * 512, 512)],
                                start=True,
                                stop=True,
                            )
                        ptile = pt_pool.tile(
                            [P, 1024], BF16, tag="pt", name=f"pt{qh}{p}{hh}_{j}"
                        )
                        nc.scalar.activation(ptile[:], sps[:], EXPF)
                        for nn in range(2):
                            nc.tensor.matmul(
                                cps[:, bass.ts(nn, 512)],
                                v_t[j][:, head, :],
                                ptile[:, bass.ts(nn, 512)],
                                start=(j == 0),
                                stop=(j == LT - 1),
                            )
                    emit_norm(qh, p, hh, cps)

                kq1 = [
                    (lambda n=n: emit_kq1("k", n)) for n in range(NQ)
                ] + [(lambda n=n: emit_kq1("q", n)) for n in range(NQ)]
                v_fill = [(lambda lt=lt: emit_v(lt)) for lt in range(LT)]
                c0 = [(lambda lt=lt: emit_c(lt)) for lt in range(LT // 2)]
                c1 = [(lambda lt=lt: emit_c(lt)) for lt in range(LT // 2, LT)]

                # windows: pair 0 first (its K/Q ready), V inside window 0,
                # K1/Q1 spread over windows 1-3, C(qh0) into windows 6-7.
                emit_window(0, 0, 0, v_fill)
                emit_window(0, 0, 1, kq1[0:3])
                emit_window(1, 0, 0, kq1[3:6])
                emit_window(1, 0, 1, kq1[6:8])
                emit_window(0, 1, 0, [])
                emit_window(0, 1, 1, [])
                emit_window(1, 1, 0, c0[0:6])
                emit_window(1, 1, 1, c0[6:8])
                for f in c1:
                    f()

    return nc


_NC = None


def _get_nc():
    global _NC
    if _NC is None:
        _NC = build_nc()
    return _NC


def kernel(query, key, value, w_in, b_in, w_out, b_out):
    import ml_dtypes

    bf16 = ml_dtypes.bfloat16
    query = np.asarray(query, dtype=np.float32)
    key = np.asarray(key, dtype=np.float32)
    value = np.asarray(value, dtype=np.float32)
    w_in = np.asarray(w_in, dtype=np.float32)
    b_in = np.asarray(b_in, dtype=np.float32)
    w_out = np.asarray(w_out, dtype=np.float32)
    b_out = np.asarray(b_out, dtype=np.float32)

    scale = float(D) ** -0.5
    in_maps = []
    for c in range(NCORES):
        b = c % 2
        g = c // 2
        sl = slice(FL * g, FL * (g + 1))
        wq = w_in[0 * E : 1 * E][sl] * scale  # (256, 1024)
        wk = w_in[1 * E : 2 * E][sl]
        wv = w_in[2 * E : 3 * E][sl]
        in_maps.append(
            {
                "xq_t": np.ascontiguousarray(query[:, b, :].T).astype(bf16),
                "xk_t": np.ascontiguousarray(key[:, b, :].T).astype(bf16),
                "xv_t": np.ascontiguousarray(value[:, b, :].T).astype(bf16),
                "wq_t": np.ascontiguousarray(wq.T).astype(bf16),
                "wk_t": np.ascontiguousarray(wk.T).astype(bf16),
                "wv_t": np.ascontiguousarray(wv.T).astype(bf16),
                "wo_t": np.ascontiguousarray(w_out[:, sl].T).astype(bf16),
                "bq": np.ascontiguousarray(b_in[0 * E : 1 * E][sl] * scale),
                "bk": np.ascontiguousarray(b_in[1 * E : 2 * E][sl]),
                "bv": np.ascontiguousarray(b_in[2 * E : 3 * E][sl]),
                "bo": b_out if c < 2 else np.zeros_like(b_out),
            }
        )

    nc = _get_nc()
    res = run_bass_kernel_spmd(
        nc, in_maps, list(range(NCORES)), trace=TRACE, **TRACE_KWARGS
    )
    global LAST_RESULTS
    LAST_RESULTS = res

    out = np.zeros((L, B, E), dtype=np.float32)
    for c in range(NCORES):
        out[:, c % 2, :] += res.results[c]["out_p"]
    return out


# revision 5
# speedup vs baseline: 1.0338x; 1.0338x over previous
"""Trainium2 Bass kernel for DPMultiheadAttention (L=2048, B=2, E=1024, H=16).

Sharding: batch*head parallel across 8 cores. Core c handles batch c%2 and
heads [4*(c//2), 4*(c//2)+4). Each core computes q/k/v projections for its
256-feature slice, per-head attention, and a partial out-projection; the host
sums the per-batch partials.

Software-pipelined schedule (v2): the kernel is one long interleaved stream
instead of serial phases. The exp stream on the Scalar engine is the
second-longest resource (~136us busy) after PE (~164us), so:
  - xk/xq/xv are DMAed in parallel on the three DGE queues (scalar/sync/
    gpsimd) so K0/Q0 projections finish ~25us in and the first scores/exp
    start there, not at 80us.
  - Attention runs as 8 windows (qh, pair, head): scores -> exp -> ctx per
    128-key chunk j. Remaining projection work (V, K1/Q1) and the qh0
    out-projection are interleaved into the windows as PE filler so the PE
    queue never idles while the Scalar engine works through the exps.
  - PSUM: scores double-buffer (4 banks) + one ctx accumulator (2 banks) +
    a shared 2-buf pool for projections/out-proj (2 banks) = 8 banks.
  - Softmax denominators ride as a ones-column in the padded V operand;
    normalization (reciprocal spread over 4 partitions, DMA row-broadcast)
    happens per window off the critical path; out-projection per 128-token
    chunk follows once all four heads' ctx for that qh are normalized.
"""

import numpy as np

import concourse.bass as bass
import concourse.tile as tile
from concourse import mybir
from concourse.bass_utils import run_bass_kernel_spmd

L = 2048
B = 2
E = 1024
H = 16
D = 64
NCORES = 8
HPC = H // NCORES * B  # heads per core = 4
FL = HPC * D  # local feature slice = 256
P = 128

BF16 = mybir.dt.bfloat16
FP32 = mybir.dt.float32

TRACE = False
TRACE_KWARGS = {}
LAST_RESULTS = None


class PatchedTileContext(tile.TileContext):
    """This walrus build caps sync-wait slots per instruction at one; Tile's
    sem assigner freely attaches several. Split extra waits onto same-engine
    nops inserted just before the owning instruction."""

    MAX_WAITS = 1

    def _split_inst_waits(self, inst, out_list):
        si = getattr(inst, "sync_info", None)
        if si is not None and len(si.on_wait) > self.MAX_WAITS:
            waits = list(si.on_wait)
            keep = len(waits) - self.MAX_WAITS
            for i in range(0, keep, self.MAX_WAITS):
                out_list.append(
                    mybir.InstNoOp(
                        name=f"I-ws-{self.nc.next_id()}",
                        engine=inst.engine,
                        bass_nofuse=True,
                        sync_info=mybir.SyncInfo(
                            on_wait=waits[i : i + self.MAX_WAITS], on_update=[]
                        ),
                    )
                )
            inst.sync_info = mybir.SyncInfo(
                on_wait=waits[keep:], on_update=list(si.on_update)
            )
        out_list.append(inst)

    def _lower_ordered_insts(self, ordered):
        for insts in ordered.values():
            new_list = []
            for inst in insts:
                self._split_inst_waits(inst, new_list)
            insts[:] = new_list
        super()._lower_ordered_insts(ordered)

    def _drain_and_barrier(self, tick_clock, wait_clock):
        from bass_rust import SyncInfo
        from concourse.vector_clock import ScopedClock

        drain_inst = self.nc.sync.drain()
        wait_clock.add_sem_waits(
            drain_inst.ins, ScopedClock({None: tick_clock.global_clock})
        )
        si = drain_inst.ins.sync_info
        if si is not None and len(si.on_wait) > self.MAX_WAITS:
            waits = list(si.on_wait)
            drain_inst.ins.sync_info = SyncInfo(
                on_wait=waits[: self.MAX_WAITS], on_update=list(si.on_update)
            )
            for i in range(self.MAX_WAITS, len(waits), self.MAX_WAITS):
                nop = self.nc.sync.nop(nofuse=True)
                nop.ins.sync_info = SyncInfo(
                    on_wait=waits[i : i + self.MAX_WAITS], on_update=[]
                )

        self.nc.all_engine_barrier()
        assert self.sems is not None
        popped = self.nc._tile_sem_poison_stack.pop()
        assert popped is self._sem_poison
        self.nc.clear_and_free_semaphores(list(self.sems.allocated().values()))
        self.nc.all_engine_barrier()


def _ap3(ap, dims):
    return bass.AP(tensor=ap.tensor, offset=ap.offset, ap=dims)


def _bcast_ap(t):
    """DRAM 1-D tensor -> (128, len) partition-broadcast AP for DMA."""
    ap = t[:]
    return bass.AP(tensor=ap.tensor, offset=ap.offset, ap=[[0, P], *ap.ap])


KT = E // P  # 8 contraction tiles for projections
MT = FL // P  # 2 feature tiles (pairs)
NQ = L // 512  # 4 token chunks of 512
LT = L // P  # 16 token tiles of 128
EXPF = mybir.ActivationFunctionType.Exp


def build_nc():
    nc = bass.Bass()

    xq = nc.declare_dram_parameter("xq_t", [E, L], BF16, isOutput=False)
    xk = nc.declare_dram_parameter("xk_t", [E, L], BF16, isOutput=False)
    xv = nc.declare_dram_parameter("xv_t", [E, L], BF16, isOutput=False)
    wq = nc.declare_dram_parameter("wq_t", [E, FL], BF16, isOutput=False)
    wk = nc.declare_dram_parameter("wk_t", [E, FL], BF16, isOutput=False)
    wv = nc.declare_dram_parameter("wv_t", [E, FL], BF16, isOutput=False)
    wo = nc.declare_dram_parameter("wo_t", [FL, E], BF16, isOutput=False)
    bq = nc.declare_dram_parameter("bq", [FL], FP32, isOutput=False)
    bk = nc.declare_dram_parameter("bk", [FL], FP32, isOutput=False)
    bv = nc.declare_dram_parameter("bv", [FL], FP32, isOutput=False)
    bo = nc.declare_dram_parameter("bo", [E], FP32, isOutput=False)
    out = nc.declare_dram_parameter("out_p", [L, E], BF16, isOutput=True)

    with PatchedTileContext(nc) as tc:
        with (
            tc.tile_pool(name="singles", bufs=1) as singles,
            tc.tile_pool(name="pt", bufs=6) as pt_pool,
            tc.tile_pool(name="norm", bufs=2) as norm_pool,
            tc.tile_pool(name="outsb", bufs=4) as out_pool,
            tc.tile_pool(name="xvg", bufs=2) as xvg_pool,
        ):
            # ---- activation-table preload: tiny exp before anything else ----
            dummy = singles.tile([1, 32], FP32, tag="dummy")
            nc.vector.memset(dummy[:], 1.0)
            nc.scalar.activation(dummy[:], dummy[:], EXPF)

            # ---- weights / biases ----
            wq_sb = singles.tile([P, KT, FL], BF16, tag="wq")
            wk_sb = singles.tile([P, KT, FL], BF16, tag="wk")
            wv_sb = singles.tile([P, KT, FL], BF16, tag="wv")
            wo_sb = singles.tile([P, MT, E], BF16, tag="wo")
            bq_sb = singles.tile([P, MT], FP32, tag="bq")
            bk_sb = singles.tile([P, MT], FP32, tag="bk")
            bv_sb = singles.tile([P, FL], FP32, tag="bv")
            bo_sb = singles.tile([P, E], FP32, tag="bo")

            # ---- activations (inputs) ----
            xq_sb = singles.tile([P, KT, L], BF16, tag="xq")
            xk_sb = singles.tile([P, KT, L], BF16, tag="xk")

            # DMA: everything on the sync DGE (multiple HW queues run in
            # parallel at full HBM BW), ordered by when compute needs it:
            # K/Q weights+inputs first (scores path), then xv token-groups
            # feeding a 2-deep ring, then the out-proj weights.
            nc.sync.dma_start(wk_sb[:], wk.rearrange("(o p) f -> p o f", p=P))
            nc.sync.dma_start(bk_sb[:], bk.rearrange("(o p) -> p o", p=P))
            nc.sync.dma_start(wq_sb[:], wq.rearrange("(o p) f -> p o f", p=P))
            nc.sync.dma_start(bq_sb[:], bq.rearrange("(o p) -> p o", p=P))
            nc.sync.dma_start(wv_sb[:], wv.rearrange("(o p) f -> p o f", p=P))
            nc.sync.dma_start(bv_sb[:], _bcast_ap(bv))
            xk_re = xk.rearrange("(o p) m -> p o m", p=P)
            xq_re = xq.rearrange("(o p) m -> p o m", p=P)
            xv_re = xv.rearrange("(o p) m -> p o m", p=P)
            for c4 in range(4):
                nc.sync.dma_start(
                    xk_sb[:, 2 * c4 : 2 * c4 + 2, :], xk_re[:, 2 * c4 : 2 * c4 + 2, :]
                )
            for c4 in range(4):
                nc.sync.dma_start(
                    xq_sb[:, 2 * c4 : 2 * c4 + 2, :], xq_re[:, 2 * c4 : 2 * c4 + 2, :]
                )

            # ---- persistent activations ----
            # Q^T zero-padded per head: within pair tile, head hh lives in
            # partition rows [64*hh, 64*hh+64); other rows 0.
            qtp_t = [
                singles.tile([P, 2, L], BF16, tag=f"qtp{p}", name=f"qtp{p}")
                for p in range(MT)
            ]
            kt_t = [
                singles.tile([P, L], BF16, tag=f"kt{p}", name=f"kt{p}")
                for p in range(MT)
            ]
            # V padded per head to 128 cols: [V_h (64) | ones | zeros(63)]
            v_t = [
                singles.tile([P, HPC, P], BF16, tag=f"v{j}", name=f"v{j}")
                for j in range(LT)
            ]
            ctx_t = [
                singles.tile([P, L], BF16, tag=f"ctx{p}", name=f"ctx{p}")
                for p in range(MT)
            ]

            for p in range(MT):
                nc.vector.memset(qtp_t[p][D:P, 0, :], 0.0)
                nc.vector.memset(qtp_t[p][0:D, 1, :], 0.0)
            for j in range(LT):
                nc.vector.memset(v_t[j][:, :, D : D + 1], 1.0)
                nc.vector.memset(v_t[j][:, :, D + 1 : P], 0.0)

            # xv ring: 4 token-groups of 512 tokens, 2 resident at a time
            xv_ring = [
                xvg_pool.tile([P, KT, 512], BF16, tag="xvg", name=f"xvg{g}")
                for g in range(2)
            ]
            for g in range(2):
                nc.sync.dma_start(xv_ring[g][:], xv_re[:, :, bass.ts(g, 512)])
            nc.sync.dma_start(wo_sb[:], wo.rearrange("(o p) f -> p o f", p=P))
            nc.sync.dma_start(bo_sb[:], _bcast_ap(bo))

            # ================= K0/Q0 projections (pair 0) =================
            with tc.tile_pool(name="p0", bufs=4, space="PSUM") as p0_pool:
                for which, w_sb, x_sb, b_sb in (
                    ("k0", wk_sb, xk_sb, bk_sb),
                    ("q0", wq_sb, xq_sb, bq_sb),
                ):
                    ps = [
                        p0_pool.tile([P, 512], FP32, tag="p0", name=f"ps{which}{n}")
                        for n in range(NQ)
                    ]
                    for k in range(KT):
                        for n in range(NQ):
                            nc.tensor.matmul(
                                ps[n][:],
                                w_sb[:, k, 0:P],
                                x_sb[:, k, bass.ts(n, 512)],
                                start=(k == 0),
                                stop=(k == KT - 1),
                            )
                    for n in range(NQ):
                        if which == "k0":
                            nc.vector.tensor_scalar_add(
                                kt_t[0][:, bass.ts(n, 512)], ps[n][:], b_sb[:, 0:1]
                            )
                        else:
                            nc.vector.tensor_scalar_add(
                                qtp_t[0][0:D, 0, bass.ts(n, 512)],
                                ps[n][0:D],
                                b_sb[0:D, 0:1],
                            )
                            nc.vector.tensor_scalar_add(
                                qtp_t[0][D:P, 1, bass.ts(n, 512)],
                                ps[n][D:P],
                                b_sb[D:P, 0:1],
                            )

            # ================= interleaved attention + rest =================
            with (
                tc.tile_pool(name="s_psum", bufs=2, space="PSUM") as s_pool,
                tc.tile_pool(name="c_psum", bufs=1, space="PSUM") as c_pool,
                tc.tile_pool(name="pa", bufs=2, space="PSUM") as pa_pool,
            ):

                xv_tiles = {0: xv_ring[0], 1: xv_ring[1]}

                def emit_v(lt):
                    g = lt // 4
                    ps = pa_pool.tile([P, 512], FP32, tag="pa", name=f"psv{lt}")
                    for k in range(KT):
                        nc.tensor.matmul(
                            ps[:, :FL],
                            xv_tiles[g][:, k, bass.ts(lt % 4, P)],
                            wv_sb[:, k, :],
                            start=(k == 0),
                            stop=(k == KT - 1),
                        )
                    nc.vector.tensor_add(
                        v_t[lt][:, :, 0:D],
                        ps[:, :FL].rearrange("p (h d) -> p h d", d=D),
                        bv_sb.rearrange("p (h d) -> p h d", d=D),
                    )
                    if lt % 4 == 3 and g + 2 < 4:
                        # group g fully consumed; stage group g+2 in its slot
                        nt = xvg_pool.tile(
                            [P, KT, 512], BF16, tag="xvg", name=f"xvg{g + 2}"
                        )
                        xv_tiles[g + 2] = nt
                        nc.sync.dma_start(nt[:], xv_re[:, :, bass.ts(g + 2, 512)])

                def emit_kq1(which, n):
                    w_sb, x_sb, b_sb = (
                        (wk_sb, xk_sb, bk_sb) if which == "k" else (wq_sb, xq_sb, bq_sb)
                    )
                    ps = pa_pool.tile([P, 512], FP32, tag="pa", name=f"ps{which}1{n}")
                    for k in range(KT):
                        nc.tensor.matmul(
                            ps[:],
                            w_sb[:, k, bass.ts(1, P)],
                            x_sb[:, k, bass.ts(n, 512)],
                            start=(k == 0),
                            stop=(k == KT - 1),
                        )
                    if which == "k":
                        nc.vector.tensor_scalar_add(
                            kt_t[1][:, bass.ts(n, 512)], ps[:], b_sb[:, 1:2]
                        )
                    else:
                        nc.vector.tensor_scalar_add(
                            qtp_t[1][0:D, 0, bass.ts(n, 512)],
                            ps[0:D],
                            b_sb[0:D, 1:2],
                        )
                        nc.vector.tensor_scalar_add(
                            qtp_t[1][D:P, 1, bass.ts(n, 512)],
                            ps[D:P],
                            b_sb[D:P, 1:2],
                        )

                def emit_c(lt):
                    osb = out_pool.tile([P, E], BF16, tag="osb", name=f"osb{lt}")
                    for nn in range(2):
                        ps = pa_pool.tile(
                            [P, 512], FP32, tag="pa", name=f"psc{lt}_{nn}"
                        )
                        for kt_i in range(MT):
                            nc.tensor.matmul(
                                ps[:],
                                ctx_t[kt_i][:, bass.ts(lt, P)],
                                wo_sb[:, kt_i, bass.ts(nn, 512)],
                                start=(kt_i == 0),
                                stop=(kt_i == MT - 1),
                            )
                        nc.vector.tensor_add(
                            osb[:, bass.ts(nn, 512)], ps[:], bo_sb[:, bass.ts(nn, 512)]
                        )
                    nc.sync.dma_start(out[bass.ts(lt, P), :], osb[:])

                def emit_norm(qh, p, hh, cps):
                    craw = norm_pool.tile(
                        [D + 1, 1024], FP32, tag="craw", name=f"craw{qh}{p}{hh}"
                    )
                    nc.scalar.copy(craw[:], cps[0 : D + 1, :])
                    # reciprocal of the sums row, spread over 4 32-aligned
                    # partitions so it runs 4 DVE lanes wide instead of 1
                    rt = norm_pool.tile([97, 256], FP32, tag="rt", name=f"rt{qh}{p}{hh}")
                    for k4 in range(4):
                        nc.vector.tensor_copy(
                            rt[32 * k4 : 32 * k4 + 1, :],
                            craw[D : D + 1, bass.ts(k4, 256)],
                        )
                    nc.vector.reciprocal(rt[:], rt[:])
                    rrow = norm_pool.tile([1, 1024], FP32, tag="rrow", name=f"rrow{qh}{p}{hh}")
                    for k4 in range(4):
                        nc.vector.tensor_copy(
                            rrow[0:1, bass.ts(k4, 256)],
                            rt[32 * k4 : 32 * k4 + 1, :],
                        )
                    rb = norm_pool.tile([D, 1024], FP32, tag="rb", name=f"rb{qh}{p}{hh}")
                    rap = rrow[0:1, :]
                    nc.sync.dma_start(
                        out=_ap3(rb[:], [rb[:].ap[0], [1, 1], rb[:].ap[1]]),
                        in_=_ap3(rap, [[1, 1], [0, D], rap.ap[-1]]),
                    )
                    nc.vector.tensor_mul(
                        ctx_t[p][D * hh : D * hh + D, bass.ds(qh * 1024, 1024)],
                        craw[0:D, :],
                        rb[:],
                    )

                def emit_window(qh, p, hh, fillers):
                    head = 2 * p + hh
                    cps = c_pool.tile(
                        [P, 1024], FP32, tag="c", name=f"cps{qh}{p}{hh}"
                    )
                    done = 0
                    nf = len(fillers)
                    for j in range(LT):
                        want = (j + 1) * nf // LT
                        while done < want:
                            fillers[done]()
                            done += 1
                        sps = s_pool.tile(
                            [P, 1024], FP32, tag="s", name=f"sps{qh}{p}{hh}_{j}"
                        )
                        for nn in range(2):
                            nc.tensor.matmul(
                                sps[:, bass.ts(nn, 512)],
                                kt_t[p][:, bass.ts(j, P)],
                                qtp_t[p][:, hh, bass.ds(qh * 1024 + nn * 512, 512)],
                                start=True,
                                stop=True,
                            )
                        ptile = pt_pool.tile(
                            [P, 1024], BF16, tag="pt", name=f"pt{qh}{p}{hh}_{j}"
                        )
                        nc.scalar.activation(ptile[:], sps[:], EXPF)
                        for nn in range(2):
                            nc.tensor.matmul(
                                cps[:, bass.ts(nn, 512)],
                                v_t[j][:, head, :],
                                ptile[:, bass.ts(nn, 512)],
                                start=(j == 0),
                                stop=(j == LT - 1),
                            )
                    emit_norm(qh, p, hh, cps)

                kq1 = [
                    (lambda n=n: emit_kq1("k", n)) for n in range(NQ)
                ] + [(lambda n=n: emit_kq1("q", n)) for n in range(NQ)]
                v_fill = [(lambda lt=lt: emit_v(lt)) for lt in range(LT)]
                c0 = [(lambda lt=lt: emit_c(lt)) for lt in range(LT // 2)]
                c1 = [(lambda lt=lt: emit_c(lt)) for lt in range(LT // 2, LT)]

                # windows: pair 0 first (its K/Q ready), V inside window 0,
                # K1/Q1 spread over windows 1-3, C(qh0) into windows 6-7.
                emit_window(0, 0, 0, v_fill)
                emit_window(0, 0, 1, kq1[0:3])
                emit_window(1, 0, 0, kq1[3:6])
                emit_window(1, 0, 1, kq1[6:8])
                emit_window(0, 1, 0, [])
                emit_window(0, 1, 1, [])
                emit_window(1, 1, 0, c0[0:6])
                emit_window(1, 1, 1, c0[6:8])
                for f in c1:
                    f()

    return nc


_NC = None


def _get_nc():
    global _NC
    if _NC is None:
        _NC = build_nc()
    return _NC


def kernel(query, key, value, w_in, b_in, w_out, b_out):
    import ml_dtypes

    bf16 = ml_dtypes.bfloat16
    query = np.asarray(query, dtype=np.float32)
    key = np.asarray(key, dtype=np.float32)
    value = np.asarray(value, dtype=np.float32)
    w_in = np.asarray(w_in, dtype=np.float32)
    b_in = np.asarray(b_in, dtype=np.float32)
    w_out = np.asarray(w_out, dtype=np.float32)
    b_out = np.asarray(b_out, dtype=np.float32)

    scale = float(D) ** -0.5
    in_maps = []
    for c in range(NCORES):
        b = c % 2
        g = c // 2
        sl = slice(FL * g, FL * (g + 1))
        wq = w_in[0 * E : 1 * E][sl] * scale  # (256, 1024)
        wk = w_in[1 * E : 2 * E][sl]
        wv = w_in[2 * E : 3 * E][sl]
        in_maps.append(
            {
                "xq_t": np.ascontiguousarray(query[:, b, :].T).astype(bf16),
                "xk_t": np.ascontiguousarray(key[:, b, :].T).astype(bf16),
                "xv_t": np.ascontiguousarray(value[:, b, :].T).astype(bf16),
                "wq_t": np.ascontiguousarray(wq.T).astype(bf16),
                "wk_t": np.ascontiguousarray(wk.T).astype(bf16),
                "wv_t": np.ascontiguousarray(wv.T).astype(bf16),
                "wo_t": np.ascontiguousarray(w_out[:, sl].T).astype(bf16),
                "bq": np.ascontiguousarray(b_in[0 * E : 1 * E][sl] * scale),
                "bk": np.ascontiguousarray(b_in[1 * E : 2 * E][sl]),
                "bv": np.ascontiguousarray(b_in[2 * E : 3 * E][sl]),
                "bo": b_out if c < 2 else np.zeros_like(b_out),
            }
        )

    nc = _get_nc()
    res = run_bass_kernel_spmd(
        nc, in_maps, list(range(NCORES)), trace=TRACE, **TRACE_KWARGS
    )
    global LAST_RESULTS
    LAST_RESULTS = res

    out = np.zeros((L, B, E), dtype=np.float32)
    for c in range(NCORES):
        out[:, c % 2, :] += np.asarray(res.results[c]["out_p"], dtype=np.float32)
    return out
